# revision 27
# baseline (speedup 1.0000x reference)
"""HMM forward (negative log-marginal) on 8 TRN2 NeuronCores.

Forward-backward split: p(x) = sum_z alpha_m[z] * beta_m[z], so the 255
sequential recurrence steps become TWO independent chains of ~128 steps
run concurrently on each core:
    fwd:  a_t = eobs_t (*) (W^T a_{t-1}),   t = 1..127    (alpha)
    bwd:  b_{t-1} = W (eobs_t (*) b_t),     t = 255..128  (beta)
Each period executes one step of each chain (32 LDW+MM pairs); each
chain's psum->DVE->SBUF round trip (~480ns latency) hides under the
other chain's MM block, so the period is LDW-stream-bound (~870ns for
two steps) instead of latency-bound (767ns for one step).

Both chains run in linear space with a constant per-step rescale
exp(SHIFT); 127 fwd + 128 bwd multiplies make the final constant the
same 255*SHIFT as the pure-forward version.  Weights are fp8e4 with a
power-of-2 scale s folded into eobs (exactly cancels in the result).

Sharding: data-parallel over batch (64 -> 8 per core); W and W^T
replicated (fp8, 2KB/partition each); eobs resident in SBUF.
"""

import numpy as np
import ml_dtypes

Z = 512
X = 10000
SEQ = 256
B = 64
NCORES = 8
BS = B // NCORES  # 8 batch per core
P = 128
ZC = Z // P  # 4 z-chunks
SHIFT = 9.2
M = 127       # fwd computes a_1..a_M (M steps); bwd does 255-M = 128 steps
T0 = 16       # head eobs chunks (fwd rows 0..T0-1, bwd rows 255-T0..254)

_NC_CACHE = {}


def _build_nc():
    if "nc" in _NC_CACHE:
        return _NC_CACHE["nc"]
    from concourse import bacc
    import concourse.mybir as mybir
    import concourse.tile as tile

    bf16 = mybir.dt.bfloat16
    fp8 = mybir.dt.float8e4
    f32 = mybir.dt.float32

    nc = bacc.Bacc("TRN2", target_bir_lowering=False, debug=False,
                   num_devices=NCORES)

    # weights in device layout (host shuffles): one DMA each
    w_d = nc.dram_tensor("w", [P, ZC * Z], fp8, kind="ExternalInput")
    wt_d = nc.dram_tensor("wt", [P, ZC * Z], fp8, kind="ExternalInput")
    eobs_d = nc.dram_tensor("eobs", [P, SEQ - 1, ZC, BS], bf16,
                            kind="ExternalInput")
    ae0_d = nc.dram_tensor("ae0", [P, ZC, BS], bf16, kind="ExternalInput")
    out_d = nc.dram_tensor("out", [1, BS], f32, kind="ExternalOutput")

    from concourse.tile_rust import add_dep_helper

    NB = SEQ - 1 - M  # 128 bwd MM-steps

    with tile.TileContext(nc) as tc:
        with (
            tc.tile_pool(name="constp", bufs=1) as constp,
            tc.tile_pool(name="aep", bufs=2) as aep,
            tc.tile_pool(name="psp", bufs=1, space="PSUM") as psp,
            tc.tile_pool(name="finp", bufs=1) as finp,
        ):
            # fwd stationary: w_sb[p, ic, j] = W[ic*128+p, j]
            w_sb = constp.tile([P, ZC, Z], fp8, name="w_sb")
            nc.sync.dma_start(out=w_sb[:], in_=w_d[:])
            # bwd stationary: wt_sb[p, jc, i] = W[i, jc*128+p]
            wt_sb = constp.tile([P, ZC, Z], fp8, name="wt_sb")
            nc.sync.dma_start(out=wt_sb[:], in_=wt_d[:])

            ae_init = constp.tile([P, ZC, BS], bf16, name="ae_init")
            nc.sync.dma_start(out=ae_init[:], in_=ae0_d[:])

            # eobs: two small head tiles (fwd start rows, bwd start rows)
            # first so both chains ungate early; bulk streams under compute.
            eobs_f0 = constp.tile([P, T0, ZC, BS], bf16, name="eobs_f0")
            nc.sync.dma_start(out=eobs_f0[:], in_=eobs_d[:, 0:T0, :, :])
            eobs_b0 = constp.tile([P, T0, ZC, BS], bf16, name="eobs_b0")
            nc.sync.dma_start(out=eobs_b0[:],
                              in_=eobs_d[:, SEQ - 1 - T0:SEQ - 1, :, :])
            eobs_mid = constp.tile([P, SEQ - 1 - 2 * T0, ZC, BS], bf16,
                                   name="eobs_mid")
            nc.sync.dma_start(out=eobs_mid[:],
                              in_=eobs_d[:, T0:SEQ - 1 - T0, :, :])

            def erow(r):
                if r < T0:
                    return eobs_f0, r
                if r >= SEQ - 1 - T0:
                    return eobs_b0, r - (SEQ - 1 - T0)
                return eobs_mid, r - T0

            ones_sb = constp.tile([P, 1], bf16, name="ones_sb")
            nc.vector.memset(ones_sb[:], 1.0)
            # Load the Ln table set early so the final log doesn't stall.
            scratch = finp.tile([P, 1], f32, name="scratch")
            nc.scalar.activation(scratch[:], ones_sb[:],
                                 mybir.ActivationFunctionType.Ln)

            # slots 1-4: pair-A writes reading chunks {0,1} (gated by the
            # pair's first TT); slots 5-8: pair-A writes reading {2,3}
            # (gated by the second TT) -> pair A completes at slot 8;
            # slots 9-16: pair-B writes, all gates already satisfied.
            ORDER = [(0, 0), (1, 0), (0, 1), (1, 1),
                     (0, 2), (1, 2), (0, 3), (1, 3),
                     (2, 0), (3, 0), (2, 1), (3, 1),
                     (2, 2), (3, 2), (2, 3), (3, 3)]
            first_slot = {}
            last_slot = {}
            for s, (jc, ic) in enumerate(ORDER):
                first_slot.setdefault(jc, s)
                last_slot[jc] = s

            state = {"mm": None, "tt": None}

            def mm_block(wsb, prev, ps, tag):
                for s, (jc, ic) in enumerate(ORDER):
                    m = nc.tensor.matmul(
                        ps[:, jc, 0:BS],
                        wsb[:, ic, jc * P:(jc + 1) * P],
                        prev[ic],
                        start=(s == first_slot[jc]),
                        stop=(s == last_slot[jc]),
                        skip_group_check=True,
                    )
                    if state["mm"] is not None:
                        add_dep_helper(m.ins, state["mm"], sync=False,
                                       reason=f"mm-{tag}")
                    state["mm"] = m.ins

            def tt_one(out, ps, op, tag):
                # single wide evac-multiply per chain step: one DVE entry +
                # one release per period per chain (vs two), and every chunk
                # read of the next block gates on the same TT (no mid-block
                # stall).  Tile-granular psum tracking makes the TT wait all
                # 16 MMs, which is the true dependency here anyway.
                tt = nc.vector.tensor_mul(out[:], ps[:, :, 0:BS], op)
                if state["tt"] is not None:
                    add_dep_helper(tt.ins, state["tt"], sync=False,
                                   reason=f"tt-{tag}")
                state["tt"] = tt.ins

            prev_f = [ae_init[:, ic, :] for ic in range(ZC)]
            prev_b = [eobs_b0[:, T0 - 1, ic, :] for ic in range(ZC)]
            aeA = aeB = None

            ae = None
            for per in range(1, NB + 1):
                # ---- forward chain: step `per` (only M of them) ----
                if per <= M:
                    psF = psp.tile([P, 4, 512], f32, tag="psF",
                                   name=f"psF_{per}")
                    mm_block(w_sb, prev_f, psF, "f")
                    esb, toff = erow(per - 1)
                    ae = aep.tile([P, 4, BS], bf16, tag="ae",
                                  name=f"ae_{per}")
                    tt_one(ae, psF, esb[:, toff, :, :], "f")
                    prev_f = [ae[:, ic, :] for ic in range(ZC)]

                # ---- backward chain: step `per` -> b_{255-per} ----
                psB2 = psp.tile([P, 4, 512], f32, tag="psB2",
                                name=f"psB2_{per}")
                mm_block(wt_sb, prev_b, psB2, "b")
                cc = aep.tile([P, 4, BS], bf16, tag="cc", name=f"cc_{per}")
                if per < NB:
                    # c_{255-per} = eobs row (254-per) (*) b_{255-per}
                    esb, toff = erow(SEQ - 2 - per)
                    tt_one(cc, psB2, esb[:, toff, :, :], "b")
                else:
                    # last period: d = a_M (*) b_M  (combine the chains)
                    tt_one(cc, psB2, ae[:, :, :], "fin")
                prev_b = [cc[:, ic, :] for ic in range(ZC)]

            # Final: s[b] = sum_z d[z, b], out = -(log s - 255*SHIFT).
            psf = psp.tile([1, BS], f32, tag="psF", name="ps_fin")
            for ic in range(ZC):
                nc.tensor.matmul(psf[:], ones_sb[:], prev_b[ic],
                                 start=(ic == 0), stop=(ic == ZC - 1))
            lg = finp.tile([1, BS], f32, name="lg")
            nc.scalar.activation(lg[:], psf[:],
                                 mybir.ActivationFunctionType.Ln)
            res = finp.tile([1, BS], f32, name="res")
            nc.vector.tensor_scalar(res[:], lg[:], -1.0,
                                    float(SHIFT * (SEQ - 1)),
                                    mybir.AluOpType.mult,
                                    mybir.AluOpType.add)
            nc.sync.dma_start(out=out_d[:], in_=res[:])

    nc.compile()
    _NC_CACHE["nc"] = nc
    return nc


def _log_softmax64(x, axis):
    x = np.asarray(x, np.float64)
    m = x.max(axis=axis, keepdims=True)
    return x - m - np.log(np.exp(x - m).sum(axis=axis, keepdims=True))


def host_prep(input_ids, T, pi, emit):
    """Numpy prep: normalize params, gather per-step emissions, shard."""
    ids = np.asarray(input_ids).astype(np.int64)
    T_log = _log_softmax64(T, 0)
    pi_log = _log_softmax64(pi, 0)
    emit_log = _log_softmax64(emit, 0)
    W = np.exp(T_log).T  # [i, j] = p(j|i)
    # fp8 weights scaled by a power of two; 1/s folded into eobs so the
    # on-device 255*SHIFT constant stays exact.
    s = 2.0 ** np.floor(np.log2(60.0 / W.max()))
    obs = emit_log[ids]  # [256, 64, 512]
    eobs = np.exp(obs[1:] + SHIFT) / s  # [255, 64, 512]
    ae0 = np.exp(obs[0] + pi_log[None, :])  # [64, 512]

    bf = ml_dtypes.bfloat16
    Ws = (W * s).astype(ml_dtypes.float8_e4m3)
    # fwd layout [p, ic*512 + j] = Ws[ic*128+p, j]
    w_dev = np.ascontiguousarray(
        Ws.reshape(ZC, P, Z).transpose(1, 0, 2).reshape(P, ZC * Z))
    # bwd layout [p, jc*512 + i] = Ws[i, jc*128+p]
    wt_dev = np.ascontiguousarray(
        Ws.T.reshape(ZC, P, Z).transpose(1, 0, 2).reshape(P, ZC * Z))
    in_maps = []
    for c in range(NCORES):
        bsl = slice(c * BS, (c + 1) * BS)
        e = eobs[:, bsl, :].reshape(SEQ - 1, BS, ZC, P)
        e = np.ascontiguousarray(e.transpose(3, 0, 2, 1).astype(bf))
        a = ae0[bsl, :].reshape(BS, ZC, P)
        a = np.ascontiguousarray(a.transpose(2, 1, 0).astype(bf))
        in_maps.append({"w": w_dev, "wt": wt_dev, "eobs": e, "ae0": a})
    return in_maps


def kernel(input_ids, T, pi, emit, _trace=False):
    from concourse.bass_utils import run_bass_kernel_spmd

    nc = _build_nc()
    in_maps = host_prep(input_ids, T, pi, emit)
    r = run_bass_kernel_spmd(nc, in_maps, core_ids=list(range(NCORES)),
                             trace=_trace)
    out = np.concatenate([r.results[c]["out"][0] for c in range(NCORES)])
    if _trace:
        kernel.last_results = r
    return out.astype(np.float32)


# revision 29
# speedup vs baseline: 1.1914x; 1.1914x over previous
"""HMM forward (negative log-marginal) on 8 TRN2 NeuronCores.

Forward-backward split: p(x) = sum_z alpha_m[z] * beta_m[z], so the 255
sequential recurrence steps become TWO independent chains of ~128 steps
run concurrently on each core:
    fwd:  a_t = eobs_t (*) (W^T a_{t-1}),   t = 1..127    (alpha)
    bwd:  b_{t-1} = W (eobs_t (*) b_t),     t = 255..128  (beta)
Each period executes one step of each chain (32 LDW+MM pairs); each
chain's psum->DVE->SBUF round trip (~480ns latency) hides under the
other chain's MM block, so the period is LDW-stream-bound (~870ns for
two steps) instead of latency-bound (767ns for one step).

Both chains run in linear space with a constant per-step rescale
exp(SHIFT); 127 fwd + 128 bwd multiplies make the final constant the
same 255*SHIFT as the pure-forward version.  Weights are fp8e4 with a
power-of-2 scale s folded into eobs (exactly cancels in the result).

Sharding: data-parallel over batch (64 -> 8 per core); W and W^T
replicated (fp8, 2KB/partition each); eobs resident in SBUF.
"""

import numpy as np
import ml_dtypes

Z = 512
X = 10000
SEQ = 256
B = 64
NCORES = 8
BS = B // NCORES  # 8 batch per core
P = 128
ZC = Z // P  # 4 z-chunks
SHIFT = 9.2
M = 127       # fwd computes a_1..a_M (M steps); bwd does 255-M = 128 steps
T0 = 16       # head eobs chunks (fwd rows 0..T0-1, bwd rows 255-T0..254)

_NC_CACHE = {}


def _build_nc():
    if "nc" in _NC_CACHE:
        return _NC_CACHE["nc"]
    from concourse import bacc
    import concourse.mybir as mybir
    import concourse.tile as tile

    bf16 = mybir.dt.bfloat16
    fp8 = mybir.dt.float8e4
    f32 = mybir.dt.float32

    nc = bacc.Bacc("TRN2", target_bir_lowering=False, debug=False,
                   num_devices=NCORES)

    # weights in device layout (host shuffles): one DMA each
    w_d = nc.dram_tensor("w", [P, ZC * Z], fp8, kind="ExternalInput")
    wt_d = nc.dram_tensor("wt", [P, ZC * Z], fp8, kind="ExternalInput")
    eobs_d = nc.dram_tensor("eobs", [P, SEQ - 1, ZC, BS], bf16,
                            kind="ExternalInput")
    ae0_d = nc.dram_tensor("ae0", [P, ZC, BS], bf16, kind="ExternalInput")
    out_d = nc.dram_tensor("out", [1, BS], f32, kind="ExternalOutput")

    from concourse.tile_rust import add_dep_helper

    NB = SEQ - 1 - M  # 128 bwd MM-steps

    with tile.TileContext(nc) as tc:
        with (
            tc.tile_pool(name="constp", bufs=1) as constp,
            tc.tile_pool(name="aep", bufs=2) as aep,
            tc.tile_pool(name="psp", bufs=1, space="PSUM") as psp,
            tc.tile_pool(name="finp", bufs=1) as finp,
        ):
            # fwd stationary: w_sb[p, ic, j] = W[ic*128+p, j]
            w_sb = constp.tile([P, ZC, Z], fp8, name="w_sb")
            nc.sync.dma_start(out=w_sb[:], in_=w_d[:])
            # bwd stationary: wt_sb[p, jc, i] = W[i, jc*128+p]
            wt_sb = constp.tile([P, ZC, Z], fp8, name="wt_sb")
            nc.sync.dma_start(out=wt_sb[:], in_=wt_d[:])

            ae_init = constp.tile([P, ZC, BS], bf16, name="ae_init")
            nc.sync.dma_start(out=ae_init[:], in_=ae0_d[:])

            # eobs: two small head tiles (fwd start rows, bwd start rows)
            # first so both chains ungate early; bulk streams under compute.
            eobs_f0 = constp.tile([P, T0, ZC, BS], bf16, name="eobs_f0")
            nc.sync.dma_start(out=eobs_f0[:], in_=eobs_d[:, 0:T0, :, :])
            eobs_b0 = constp.tile([P, T0, ZC, BS], bf16, name="eobs_b0")
            nc.sync.dma_start(out=eobs_b0[:],
                              in_=eobs_d[:, SEQ - 1 - T0:SEQ - 1, :, :])
            eobs_mid = constp.tile([P, SEQ - 1 - 2 * T0, ZC, BS], bf16,
                                   name="eobs_mid")
            nc.sync.dma_start(out=eobs_mid[:],
                              in_=eobs_d[:, T0:SEQ - 1 - T0, :, :])

            def erow(r):
                if r < T0:
                    return eobs_f0, r
                if r >= SEQ - 1 - T0:
                    return eobs_b0, r - (SEQ - 1 - T0)
                return eobs_mid, r - T0

            ones_sb = constp.tile([P, 1], bf16, name="ones_sb")
            nc.vector.memset(ones_sb[:], 1.0)
            # Load the Ln table set early so the final log doesn't stall.
            scratch = finp.tile([P, 1], f32, name="scratch")
            nc.scalar.activation(scratch[:], ones_sb[:],
                                 mybir.ActivationFunctionType.Ln)

            # slots 1-4: pair-A writes reading chunks {0,1} (gated by the
            # pair's first TT); slots 5-8: pair-A writes reading {2,3}
            # (gated by the second TT) -> pair A completes at slot 8;
            # slots 9-16: pair-B writes, all gates already satisfied.
            ORDER = [(0, 0), (1, 0), (0, 1), (1, 1),
                     (0, 2), (1, 2), (0, 3), (1, 3),
                     (2, 0), (3, 0), (2, 1), (3, 1),
                     (2, 2), (3, 2), (2, 3), (3, 3)]
            first_slot = {}
            last_slot = {}
            for s, (jc, ic) in enumerate(ORDER):
                first_slot.setdefault(jc, s)
                last_slot[jc] = s

            state = {"mm": None, "tt": None}

            def mm_block(wsb, prev, pA, pB, tag):
                pspair = [pA, pB]
                for s, (jc, ic) in enumerate(ORDER):
                    m = nc.tensor.matmul(
                        pspair[jc // 2][:, jc % 2, 0:BS],
                        wsb[:, ic, jc * P:(jc + 1) * P],
                        prev[ic],
                        start=(s == first_slot[jc]),
                        stop=(s == last_slot[jc]),
                        skip_group_check=True,
                    )
                    if state["mm"] is not None:
                        add_dep_helper(m.ins, state["mm"], sync=False,
                                       reason=f"mm-{tag}")
                    state["mm"] = m.ins
                # keep the PE sequencer retiring past the last MM's drain so
                # its psum-complete semaphore increment posts before the NX
                # blocks on the next block's gated head LDW (the evac TT
                # otherwise observes the sem ~260ns late).
                for _ in range(10):
                    n = nc.tensor.nop(nofuse=True)
                    ni = getattr(n, "ins", n)
                    add_dep_helper(ni, state["mm"], sync=False,
                                   reason=f"nopf-{tag}")
                    state["mm"] = ni

            def tt_pair(outA, outB, pA, pB, opA, opB, tag):
                ttA = nc.vector.tensor_mul(outA[:], pA[:, :, 0:BS], opA)
                if state["tt"] is not None:
                    add_dep_helper(ttA.ins, state["tt"], sync=False,
                                   reason=f"ttA-{tag}")
                ttB = nc.vector.tensor_mul(outB[:], pB[:, :, 0:BS], opB)
                add_dep_helper(ttB.ins, ttA.ins, sync=False,
                               reason=f"ttB-{tag}")
                state["tt"] = ttB.ins

            prev_f = [ae_init[:, ic, :] for ic in range(ZC)]
            prev_b = [eobs_b0[:, T0 - 1, ic, :] for ic in range(ZC)]
            aeA = aeB = None

            for per in range(1, NB + 1):
                # ---- forward chain: step `per` (only M of them) ----
                if per <= M:
                    psfA = psp.tile([P, 2, 512], f32, tag="psfA",
                                    name=f"psfA_{per}")
                    psfB = psp.tile([P, 2, 512], f32, tag="psfB",
                                    name=f"psfB_{per}")
                    mm_block(w_sb, prev_f, psfA, psfB, "f")
                    esb, toff = erow(per - 1)
                    aeA = aep.tile([P, 2, BS], bf16, tag="aeA",
                                   name=f"aeA_{per}")
                    aeB = aep.tile([P, 2, BS], bf16, tag="aeB",
                                   name=f"aeB_{per}")
                    tt_pair(aeA, aeB, psfA, psfB,
                            esb[:, toff, 0:2, :], esb[:, toff, 2:4, :], "f")
                    prev_f = [aeA[:, 0, :], aeA[:, 1, :],
                              aeB[:, 0, :], aeB[:, 1, :]]

                # ---- backward chain: step `per` -> b_{255-per} ----
                psbA = psp.tile([P, 2, 512], f32, tag="psbA",
                                name=f"psbA_{per}")
                psbB = psp.tile([P, 2, 512], f32, tag="psbB",
                                name=f"psbB_{per}")
                mm_block(wt_sb, prev_b, psbA, psbB, "b")
                cA = aep.tile([P, 2, BS], bf16, tag="cA", name=f"cA_{per}")
                cB = aep.tile([P, 2, BS], bf16, tag="cB", name=f"cB_{per}")
                if per < NB:
                    # c_{255-per} = eobs row (254-per) (*) b_{255-per}
                    esb, toff = erow(SEQ - 2 - per)
                    tt_pair(cA, cB, psbA, psbB,
                            esb[:, toff, 0:2, :], esb[:, toff, 2:4, :], "b")
                else:
                    # last period: d = a_M (*) b_M  (combine the chains)
                    tt_pair(cA, cB, psbA, psbB,
                            aeA[:, :, :], aeB[:, :, :], "fin")
                prev_b = [cA[:, 0, :], cA[:, 1, :], cB[:, 0, :], cB[:, 1, :]]

            # Final: s[b] = sum_z d[z, b], out = -(log s - 255*SHIFT).
            psf = psp.tile([1, BS], f32, tag="psfA", name="ps_fin")
            for ic in range(ZC):
                nc.tensor.matmul(psf[:], ones_sb[:], prev_b[ic],
                                 start=(ic == 0), stop=(ic == ZC - 1))
            lg = finp.tile([1, BS], f32, name="lg")
            nc.scalar.activation(lg[:], psf[:],
                                 mybir.ActivationFunctionType.Ln)
            res = finp.tile([1, BS], f32, name="res")
            nc.vector.tensor_scalar(res[:], lg[:], -1.0,
                                    float(SHIFT * (SEQ - 1)),
                                    mybir.AluOpType.mult,
                                    mybir.AluOpType.add)
            nc.sync.dma_start(out=out_d[:], in_=res[:])

    nc.compile()
    _NC_CACHE["nc"] = nc
    return nc


def _log_softmax64(x, axis):
    x = np.asarray(x, np.float64)
    m = x.max(axis=axis, keepdims=True)
    return x - m - np.log(np.exp(x - m).sum(axis=axis, keepdims=True))


def host_prep(input_ids, T, pi, emit):
    """Numpy prep: normalize params, gather per-step emissions, shard."""
    ids = np.asarray(input_ids).astype(np.int64)
    T_log = _log_softmax64(T, 0)
    pi_log = _log_softmax64(pi, 0)
    emit_log = _log_softmax64(emit, 0)
    W = np.exp(T_log).T  # [i, j] = p(j|i)
    # fp8 weights scaled by a power of two; 1/s folded into eobs so the
    # on-device 255*SHIFT constant stays exact.
    s = 2.0 ** np.floor(np.log2(60.0 / W.max()))
    obs = emit_log[ids]  # [256, 64, 512]
    eobs = np.exp(obs[1:] + SHIFT) / s  # [255, 64, 512]
    ae0 = np.exp(obs[0] + pi_log[None, :])  # [64, 512]

    bf = ml_dtypes.bfloat16
    Ws = (W * s).astype(ml_dtypes.float8_e4m3)
    # fwd layout [p, ic*512 + j] = Ws[ic*128+p, j]
    w_dev = np.ascontiguousarray(
        Ws.reshape(ZC, P, Z).transpose(1, 0, 2).reshape(P, ZC * Z))
    # bwd layout [p, jc*512 + i] = Ws[i, jc*128+p]
    wt_dev = np.ascontiguousarray(
        Ws.T.reshape(ZC, P, Z).transpose(1, 0, 2).reshape(P, ZC * Z))
    in_maps = []
    for c in range(NCORES):
        bsl = slice(c * BS, (c + 1) * BS)
        e = eobs[:, bsl, :].reshape(SEQ - 1, BS, ZC, P)
        e = np.ascontiguousarray(e.transpose(3, 0, 2, 1).astype(bf))
        a = ae0[bsl, :].reshape(BS, ZC, P)
        a = np.ascontiguousarray(a.transpose(2, 1, 0).astype(bf))
        in_maps.append({"w": w_dev, "wt": wt_dev, "eobs": e, "ae0": a})
    return in_maps


def kernel(input_ids, T, pi, emit, _trace=False):
    from concourse.bass_utils import run_bass_kernel_spmd

    nc = _build_nc()
    in_maps = host_prep(input_ids, T, pi, emit)
    r = run_bass_kernel_spmd(nc, in_maps, core_ids=list(range(NCORES)),
                             trace=_trace)
    out = np.concatenate([r.results[c]["out"][0] for c in range(NCORES)])
    if _trace:
        kernel.last_results = r
    return out.astype(np.float32)
